# revision 1
# baseline (speedup 1.0000x reference)
"""Trainium2 Bass kernel for BasicAttention with softmax over the QUERY axis.

reference:
    scores = einsum("bqd,bkd->bqk", q, k)      # [B,Q,K]
    attn   = softmax(scores, axis=1)           # over q (per (b,k) column)
    out    = einsum("bqk,bkd->bqd", attn, v)   # [B,Q,D]

Shapes: B=8, Q=K=2048, D=1024, fp32.

Strategy: batch-parallel over the 8 NeuronCores (one batch element per
core). Per core everything is phrased in the transposed score layout
scoresT[k, q] so the softmax reduction runs along the free axis:

  phase 0: QT[d, q] built from q via PE transposes, stored f32r
  phase 1: per k-tile (128 rows): KT chunk via PE transposes; scoresT
           block [128k, 2048q] = KT.T @ QT accumulated over d in PSUM
           (f32r matmuls); softmax over free axis; attnT block -> bf16
  phase 2: out[q, d] = sum_kt attnT[kt].T @ V[kt] (bf16 matmuls,
           PSUM-accumulated over the 16 k-tiles)

f32r (tf32-like, 11 mantissa bits) keeps the exp() input accurate
(~2e-4 score error); attn weights are in [0,1] so bf16 is plenty for
phase 2. Measured end-to-end max rel err vs fp32 reference ~3e-3.
"""

import sys

sys.path.insert(0, "/opt/trn_rl_repo")

from contextlib import ExitStack

import numpy as np

import concourse.bass as bass
import concourse.tile as tile
from concourse import bacc, bass_utils, mybir
from concourse.masks import make_identity

B, NQ, NK, D = 8, 2048, 2048, 1024
P = 128                 # partition size
DC = D // P             # 8 d-chunks
KT_N = NK // P          # 16 k-tiles
QT_N = NQ // P          # 16 q-tiles
N_MM = 512              # matmul moving free dim (one PSUM bank)

F32 = mybir.dt.float32
F32R = mybir.dt.float32r
BF16 = mybir.dt.bfloat16

_cached = None


def _build():
    nc = bacc.Bacc("TRN2", debug=False, num_devices=B)

    q_dram = nc.dram_tensor("q", (NQ, D), F32, kind="ExternalInput").ap()
    k_dram = nc.dram_tensor("k", (NK, D), F32, kind="ExternalInput").ap()
    v_dram = nc.dram_tensor("v", (NK, D), F32, kind="ExternalInput").ap()
    out_dram = nc.dram_tensor("out", (NQ, D), F32, kind="ExternalOutput").ap()

    with tile.TileContext(nc) as tc:
        with ExitStack() as ctx:
            const_pool = ctx.enter_context(tc.tile_pool(name="const", bufs=1))
            big_pool = ctx.enter_context(tc.tile_pool(name="big", bufs=1))
            nat_pool = ctx.enter_context(tc.tile_pool(name="nat", bufs=3))
            kt_pool = ctx.enter_context(tc.tile_pool(name="ktp", bufs=2))
            sc_pool = ctx.enter_context(tc.tile_pool(name="scores", bufs=2))
            small_pool = ctx.enter_context(tc.tile_pool(name="small", bufs=4))
            out_pool = ctx.enter_context(tc.tile_pool(name="outp", bufs=2))
            tp_psum = ctx.enter_context(
                tc.tile_pool(name="tpsum", bufs=2, space="PSUM")
            )
            sc_psum = ctx.enter_context(
                tc.tile_pool(name="spsum", bufs=2, space="PSUM")
            )
            o_psum = ctx.enter_context(
                tc.tile_pool(name="opsum", bufs=2, space="PSUM")
            )

            ident = const_pool.tile([P, P], F32)
            make_identity(nc, ident[:])

            # persistent big tensors (one partition-row of tiles each)
            qt = big_pool.tile([P, DC * NQ], F32R, tag="qt")      # 64 KB/part
            attnt = big_pool.tile([P, KT_N * NQ], BF16, tag="at")  # 64 KB/part
            vt = big_pool.tile([P, KT_N * D], BF16, tag="vt")      # 32 KB/part

            def transpose_block(nat_tile, dc, dst_slice, i):
                """PE-transpose nat_tile[:, dc*128:+128] -> dst_slice (f32r/sbuf)."""
                pt = tp_psum.tile([P, P], F32, tag="tp")
                nc.tensor.transpose(pt[:], nat_tile[:, dc * P:(dc + 1) * P], ident[:])
                eng = nc.vector if i % 2 == 0 else nc.scalar
                if eng is nc.vector:
                    nc.vector.tensor_copy(dst_slice, pt[:])
                else:
                    nc.scalar.copy(dst_slice, pt[:])

            # ---- phase 0: build QT (d-major) ----
            for rt in range(QT_N):
                qnat = nat_pool.tile([P, D], F32, tag="nat")
                nc.sync.dma_start(qnat[:], q_dram[rt * P:(rt + 1) * P, :])
                for dc in range(DC):
                    transpose_block(
                        qnat, dc,
                        qt[:, dc * NQ + rt * P: dc * NQ + (rt + 1) * P],
                        rt * DC + dc,
                    )

            # ---- phase 1: scoresT + softmax per k-tile ----
            for kt in range(KT_N):
                knat = nat_pool.tile([P, D], F32, tag="nat")
                nc.sync.dma_start(knat[:], k_dram[kt * P:(kt + 1) * P, :])

                # V prefetch spread across phase 1 (cast fp32->bf16 in DMA)
                nc.gpsimd.dma_start(
                    vt[:, kt * D:(kt + 1) * D], v_dram[kt * P:(kt + 1) * P, :]
                )

                ktile = kt_pool.tile([P, D], F32R, tag="kt")
                for dc in range(DC):
                    transpose_block(
                        knat, dc, ktile[:, dc * P:(dc + 1) * P], kt * DC + dc
                    )

                # scoresT block [128k, 2048q], two PSUM halves of [128,1024]
                scores = sc_pool.tile([P, NQ], F32, tag="sc")
                for half in range(2):
                    ps = sc_psum.tile([P, 1024], F32, tag="sps")
                    for qc2 in range(2):
                        q0 = half * 1024 + qc2 * N_MM
                        for dc in range(DC):
                            nc.tensor.matmul(
                                ps[:, qc2 * N_MM:(qc2 + 1) * N_MM],
                                ktile[:, dc * P:(dc + 1) * P],
                                qt[:, dc * NQ + q0: dc * NQ + q0 + N_MM],
                                start=(dc == 0),
                                stop=(dc == DC - 1),
                            )
                    nc.vector.tensor_copy(
                        scores[:, half * 1024:(half + 1) * 1024], ps[:]
                    )

                # softmax over free axis (q), normalized attn -> bf16
                negmax = small_pool.tile([P, 1], F32, tag="nm")
                nc.vector.reduce_max(
                    negmax[:], scores[:], axis=mybir.AxisListType.X, negate=True
                )
                sums = small_pool.tile([P, 1], F32, tag="sm")
                at_slice = attnt[:, kt * NQ:(kt + 1) * NQ]
                nc.scalar.activation(
                    at_slice, scores[:], mybir.ActivationFunctionType.Exp,
                    bias=negmax[:], scale=1.0, accum_out=sums[:],
                )
                rz = small_pool.tile([P, 1], F32, tag="rz")
                nc.vector.reciprocal(rz[:], sums[:])
                nc.vector.tensor_scalar_mul(at_slice, at_slice, rz[:])

            # ---- phase 2: out[q, d] = sum_kt attnT[kt].T @ V[kt] ----
            for qt_i in range(QT_N):
                osb = out_pool.tile([P, D], F32, tag="ot")
                for dt_i in range(2):
                    po = o_psum.tile([P, N_MM], F32, tag="po")
                    for kt in range(KT_N):
                        nc.tensor.matmul(
                            po[:],
                            attnt[:, kt * NQ + qt_i * P: kt * NQ + (qt_i + 1) * P],
                            vt[:, kt * D + dt_i * N_MM: kt * D + (dt_i + 1) * N_MM],
                            start=(kt == 0),
                            stop=(kt == KT_N - 1),
                        )
                    if dt_i == 0:
                        nc.vector.tensor_copy(osb[:, dt_i * N_MM:(dt_i + 1) * N_MM], po[:])
                    else:
                        nc.scalar.copy(osb[:, dt_i * N_MM:(dt_i + 1) * N_MM], po[:])
                nc.sync.dma_start(out_dram[qt_i * P:(qt_i + 1) * P, :], osb[:])

    nc.compile()
    return nc


def _get_module():
    global _cached
    if _cached is None:
        _cached = _build()
    return _cached


def run(queries, keys, values, trace=False, trace_kwargs=None):
    """Run on 8 cores; returns (output [B,NQ,D] fp32, BassKernelResults)."""
    queries = np.ascontiguousarray(np.asarray(queries, dtype=np.float32))
    keys = np.ascontiguousarray(np.asarray(keys, dtype=np.float32))
    values = np.ascontiguousarray(np.asarray(values, dtype=np.float32))
    assert queries.shape == (B, NQ, D), queries.shape

    nc = _get_module()
    in_maps = [
        {"q": queries[b], "k": keys[b], "v": values[b]} for b in range(B)
    ]
    res = bass_utils.run_bass_kernel_spmd(
        nc, in_maps, core_ids=list(range(B)), trace=trace,
        **(trace_kwargs or {}),
    )
    out = np.stack([res.results[b]["out"] for b in range(B)], axis=0)
    return out, res


def kernel(queries, keys, values):
    out, _ = run(queries, keys, values)
    return out
